# revision 52
# baseline (speedup 1.0000x reference)
"""Binary CNN (dense_cnn) Trainium2 kernel — 8-core pure data parallel.

Network (per reference): 4 binarized convs + BN/hardtanh (+2 maxpools) + FC.
All sign()-nonlinearities are folded into per-channel threshold compares on
the raw conv accumulators (BN scale > 0 makes sign(affine(x)) a threshold op),
so the device pipeline is: conv -> (pool) -> threshold -> next conv, with the
continuous path (BN4 affine + hardtanh + FC) only at the end.

Host/device split is tuned for the axon tunnel (~76ms RTT, ~75MB/s): the
input's sign bits are packed 8-per-byte on the host (96B/sample instead of
3KB), the binary weights ship bit-packed, the FC weights ship bf16, derived
params stay device-resident across calls (checksum-invalidated), the output
returns as bf16, and the jitted shard_map executable plus the device-side
zero "output seed" buffers are cached across calls — so a steady-state call
is one pack + one tunnel round trip. conv1 consumes {0,1} bits with the
threshold absorbing the 2q-1 correction (padding decoded as q=0.5 == s=0).

Layouts: channels on SBUF partitions, (n, h, w) in the free dim. conv1 is done
as a K=10 (9 taps + zero row) matmul against a tap-skewed replica of the
packed sign bits built via a DRAM staging round-trip (even/odd w split so the
stride-2 conv becomes stride-1 gathers); 16 concurrent PE tiles. conv2/3
contract channels with the 3 w-taps as sequentially accumulated matmuls over
shifted free-dim views; conv4 contracts its 6 h-taps the same way. The FC
runs activation-stationary (lhsT = h4) so the output lands with samples on
partitions, making the final DMA coarse.
"""

import numpy as np
import ml_dtypes

import concourse.bass as bass
import concourse.bacc as bacc
import concourse.tile as tile
import concourse.mybir as mybir

F32 = mybir.dt.float32
BF16 = mybir.dt.bfloat16
F8 = mybir.dt.float8e4
U8 = mybir.dt.uint8
BN_EPS = 1e-5

N_CORES = 8
N_TOTAL = 8192
N_CORE = N_TOTAL // N_CORES  # 1024
B = 128  # samples per chunk
# DoubleRow (fp8 2x PE) is HW-valid only for non-overlapping rhs k-tile
# pairs at output partition base 0: conv4's h-tap pairs qualify; conv1/2/3's
# stride-1 tap windows crash the PE ifmap fetch (verified by probe).
DR_CONV2 = False
DR_CONV3 = False
DR_CONV4 = True
ALU = mybir.AluOpType
ACTF = mybir.ActivationFunctionType

# packed-weight column offsets within the unpacked [128, 1536] fp8 tile
# (w1l ships as fp8 directly: it has structural zero rows the bit-unpack
# cannot express, and single-partition DVE fixups are not allowed).
# conv2/3 weights are laid out for DoubleRow tap pairs: [t0|t1|t2|zero];
# the zero blocks are re-zeroed with full-partition memsets after unpack.
W2_C0, W2_C1 = 0, 256
W3_C0, W3_C1 = 256, 768
W4_C0, W4_C1 = 768, 1536
WCOLS = 1536


# ---------------------------------------------------------------------------
# host-side parameter preparation (pure numpy)
# ---------------------------------------------------------------------------
def host_prep(p):
    def s(k):
        return p[f"g{k}"] / np.sqrt(p[f"v{k}"] + BN_EPS)

    w1b = np.sign(p["w1"]).astype(np.float32)  # (32,1,1,9)
    w2b = np.sign(p["w2"]).astype(np.float32)  # (64,32,1,3)
    w3b = np.sign(p["w3"]).astype(np.float32)  # (128,64,1,3)
    w4b = np.sign(p["w4"]).astype(np.float32)  # (128,128,6,1)
    s1, s2, s3, s4 = s(1), s(2), s(3), s(4)
    thr1 = (p["m1"] - p["b1"] - p["be1"] / s1).astype(np.float32)  # (32,)
    S1 = w1b.sum(axis=(1, 2, 3)).astype(np.float32)
    # conv1 rhs is q = (sign+1)/2 in {0,1} (pads q=0.5): acc_s = 2 acc_q - S1
    thr1n = np.tile(-(thr1 + S1) / 2, 4).reshape(128, 1).astype(np.float32)
    thr2 = (p["m2"] - p["b2"] - p["be2"] / s2).astype(np.float32)  # (64,)
    S3 = w3b.sum(axis=(1, 2, 3)).astype(np.float32)
    thr3 = ((S3 - p["b3"] + p["m3"] - p["be3"] / s3) / 2).astype(np.float32)
    scale4 = s4.astype(np.float32)
    bias4 = ((p["b4"] - p["m4"]) * s4 + p["be4"]).astype(np.float32)

    # conv1 lhsT row order: even taps {0,2,4,6,8} then odd taps {1,3,5,7},
    # matching the two contiguous-partition skew DMAs; row 9 stays zero
    # (DoubleRow is ISA-invalid for conv1's 32c-based output tiles, so it
    # stays a regular K=10 matmul). Ships as fp8 so the zeros survive.
    w1l = np.zeros((128, 32), np.float32)
    tap_order = [0, 2, 4, 6, 8, 1, 3, 5, 7]
    for r in range(4):
        w1l[32 * r : 32 * r + 9, :] = w1b[:, 0, 0, tap_order].T
    # DoubleRow pair layout: [t0|t1|t2|zero] per row block (pair A = taps
    # 0+1, pair B = tap 2 + structural zero absorbing the dummy rhs window)
    w2l = np.zeros((128, 256), np.float32)
    for r in range(4):
        for t in range(3):
            w2l[32 * r : 32 * r + 32, t * 64 : (t + 1) * 64] = w2b[:, :, 0, t].T
    w3l = np.zeros((128, 512), np.float32)
    for r in range(2):
        for t in range(3):
            w3l[64 * r : 64 * r + 64, t * 128 : (t + 1) * 128] = w3b[:, :, 0, t].T
    w4l = np.zeros((128, 768), np.float32)
    for h in range(6):
        w4l[:, h * 128 : (h + 1) * 128] = w4b[:, :, h, 0].T
    wfcl = np.zeros((128, 160), np.float32)
    wfc = p["wfc"].astype(np.float32)  # (10, 2048), idx = c*16+w
    for w in range(16):
        wfcl[:, w * 10 : (w + 1) * 10] = wfc[:, w::16].T  # [c, j]

    wcat = np.concatenate([w2l, w3l, w4l], axis=1)  # (128, 1536)
    pk8 = np.packbits(wcat > 0, axis=-1, bitorder="little")  # (128, 192)
    pk16 = wfcl.astype(ml_dtypes.bfloat16)  # (128, 160)
    pk32 = np.concatenate(
        [
            thr1n,
            np.tile(thr2, 2).reshape(128, 1),
            (-thr3).reshape(128, 1),
            scale4.reshape(128, 1),
            bias4.reshape(128, 1),
            np.tile(p["bfc"].astype(np.float32), (128, 1)),  # (128,10)
        ],
        axis=1,
    ).astype(np.float32)  # (128, 15)
    return {
        "pk8": pk8,
        "pk16": pk16,
        "pk32": pk32,
        "w1l8": w1l.astype(ml_dtypes.float8_e4m3),
    }


_PACK_MUL = np.uint64(0x0102040810204080)
_PACK_SCRATCH = {}


def host_pack_x(x):
    """(N,1,6,128) f32 -> (N,6,16) uint8: sign bits in plain w order.

    bit b of byte (h, t) == q at w = 8t + b; the device unpack routes even
    bits to the even staging row and odd bits to the odd row (stride-4
    views). Packing via the u64 multiply trick (~5x faster than packbits);
    persistent scratch keeps the hot path allocation-free, and the packed
    byte is read as the product's top byte so no shift pass is needed.
    """
    q = _PACK_SCRATCH.get("q")
    if q is None:
        q = np.empty(N_TOTAL * 768, np.bool_)
        _PACK_SCRATCH["q"] = q
        _PACK_SCRATCH["pk"] = np.empty(N_TOTAL * 96, np.uint8)
    pk = _PACK_SCRATCH["pk"]
    np.greater(x.reshape(-1), 0, out=q)
    u = q.view(np.uint64)
    np.multiply(u, _PACK_MUL, out=u)
    np.copyto(pk, u.view(np.uint8)[7::8])
    return pk.reshape(N_TOTAL, 6, 16)


PARAM_SPECS = [
    ("pk8", [128, 192], U8),
    ("pk16", [128, 160], BF16),
    ("pk32", [128, 15], F32),
    ("w1l8", [128, 32], F8),
]


# ---------------------------------------------------------------------------
# device program
# ---------------------------------------------------------------------------
def build_program(n_core=N_CORE, num_devices=N_CORES):
    nc = bacc.Bacc("TRN2", num_devices=num_devices)
    xq = nc.dram_tensor("xq", [n_core, 6, 16], U8, kind="ExternalInput").ap()
    params = {
        name: nc.dram_tensor(name, shape, dt, kind="ExternalInput").ap()
        for name, shape, dt in PARAM_SPECS
    }
    out = nc.dram_tensor("out", [n_core, 10], BF16, kind="ExternalOutput").ap()
    # +1 guard row: the contiguous tap-skew reads overrun the last sample
    # by up to ~74B (bytes land in SBUF slots the matmul view never reads)
    xeo_d = nc.dram_tensor("xeo_scratch", [n_core + 1, 6, 2, 72], F8).ap()

    with tile.TileContext(nc) as tc:
        _emit(nc, tc, xq, params, out, xeo_d, n_core)
    nc.compile()
    return nc


def _emit(nc, tc, xq, P, out, xeo_d, n_core):
    from contextlib import ExitStack

    ctx = ExitStack()
    chunks = n_core // B
    singles = ctx.enter_context(tc.tile_pool(name="singles", bufs=1))
    big = ctx.enter_context(tc.tile_pool(name="big", bufs=1))
    small = ctx.enter_context(tc.tile_pool(name="small", bufs=4))
    x9p = ctx.enter_context(tc.tile_pool(name="x9p", bufs=2))
    psum = ctx.enter_context(tc.tile_pool(name="psum", bufs=8, space="PSUM"))

    # ---- constants: DMA packed params, unpack weights to fp8 +-1 ----------
    pk8 = singles.tile([128, 192], U8, name="pk8_sb")
    nc.gpsimd.dma_start(out=pk8, in_=P["pk8"])
    pk16 = singles.tile([128, 160], BF16, name="pk16_sb")
    nc.gpsimd.dma_start(out=pk16, in_=P["pk16"])
    pk32 = singles.tile([128, 15], F32, name="pk32_sb")
    nc.gpsimd.dma_start(out=pk32, in_=P["pk32"])
    w1l = singles.tile([128, 32], F8, name="w1l_sb")
    nc.gpsimd.dma_start(out=w1l, in_=P["w1l8"])

    wq = singles.tile([128, WCOLS], F8, name="wq_sb")  # bits as {0,1}
    for b in range(8):
        wb = small.tile([128, 192], U8, tag="wbits")
        nc.vector.tensor_scalar(wb, pk8, 1 << b, None, ALU.bitwise_and)
        nc.vector.tensor_scalar(
            wq[:, b : b + 8 * 191 + 1 : 8], wb, 0, None, ALU.is_gt
        )
    w8 = singles.tile([128, WCOLS], F8, name="w8_sb")  # +-1
    nc.vector.tensor_scalar(w8, wq, 2.0, -1.0, ALU.mult, ALU.add)
    # structural zero blocks for the DoubleRow dummy pair slots
    nc.vector.memset(w8[:, W2_C0 + 192 : W2_C0 + 256], 0.0)
    nc.vector.memset(w8[:, W3_C0 + 384 : W3_C0 + 512], 0.0)
    w2l = w8[:, W2_C0:W2_C1]
    w3l = w8[:, W3_C0:W3_C1]
    w4l = w8[:, W4_C0:W4_C1]
    thr1n = pk32[:, 0:1]
    thr2t = pk32[:, 1:2]
    thr3n = pk32[:, 2:3]
    sc4t = pk32[:, 3:4]
    bi4t = pk32[:, 4:5]
    bfct = pk32[:, 5:15]

    for ci in range(chunks):
        n0c = ci * B
        # ---- stage A: load packed bits, unpack to q in {0,1} (pads 0.5) ---
        xqt = small.tile([128, 6, 16], U8, tag="xqt")
        nc.gpsimd.dma_start(out=xqt, in_=xq[n0c : n0c + B])
        xeo = big.tile([128, 6, 2, 72], F8, tag="xeo")
        nc.vector.memset(xeo[:, :, :, 0:2], 0.5)
        nc.vector.memset(xeo[:, :, :, 66:72], 0.5)
        for b in range(8):  # w = 8t+b -> staging row b%2, col 2 + 4t + b//2
            eo, j = b % 2, b // 2
            xb = small.tile([128, 6, 16], U8, tag="xbits")
            nc.vector.tensor_scalar(xb, xqt, 1 << b, None, ALU.bitwise_and)
            nc.vector.tensor_scalar(
                xeo[:, :, eo, 2 + j : 63 + j : 4], xb, 0, None, ALU.is_gt
            )
        nc.gpsimd.dma_start(out=xeo_d[n0c : n0c + B], in_=xeo)

        # ---- stage B: conv1 (16-tile) -> Sign (ACT) -> pool (TT max) ------
        # h1pre: per-position sign bits (+-1 fp8) for the whole chunk;
        # pooling happens on SBUF afterwards (TT cannot read two PSUM views)
        h1pre = big.tile([128, 8, 4, 6, 64], F8, tag="h1pre")
        # bulk tap-skew load: one DMA per (r, par) covers all 8 rounds as
        # contiguous 3456B runs (base offset does the tap skew); the
        # (sample, h, w<64) gather happens in the matmul's strided SBUF view.
        # 8 DMA instructions per chunk instead of 64 fragmented ones.
        x9 = x9p.tile([128, 8, 3456], F8, tag="x9")
        x9v = x9.rearrange("p n (s h r) -> p n s h r", s=4, h=6)
        for r in range(4):
            for par in range(2):  # even taps -> partitions 32r+0..5,
                src = bass.AP(  # odd taps -> partitions 32r+5..10
                    tensor=xeo_d.tensor,
                    offset=(n0c + r * 4) * 864 + 72 * par,
                    ap=[[1, 5], [13824, 8], [1, 3456]],
                )
                dst = x9[32 * r + 5 * par : 32 * r + 5 * par + 5]
                nc.sync.dma_start(out=dst, in_=src)
        for rnd in range(8):
            pp1 = [
                psum.tile([128, 384], F32, tag="pp", name=f"pp1_{rnd}_{r}")
                for r in range(4)
            ]
            for r in range(4):
                for c in range(4):
                    nc.tensor.matmul(
                        pp1[r][32 * c : 32 * c + 32],
                        lhsT=w1l[32 * r : 32 * r + 10],
                        rhs=x9v[32 * r : 32 * r + 10, rnd, c, :, 0:64],
                        start=True,
                        stop=True,
                        tile_position=(32 * r, 32 * c),
                    )
            for r in range(4):
                nc.scalar.activation(
                    h1pre[:, rnd, r],
                    pp1[r].rearrange("p (h w) -> p h w", h=6),
                    ACTF.Sign,
                    bias=thr1n,
                )
        # pool pairs along w; sign(max) == max(sign). h1b holds the 4
        # n-classes (n mod 4 == c) at partition base 32c so conv2 can run
        # 4 concurrent row-tiles.
        h1b = big.tile([128, 32, 6, 35], F8, tag="h1b")
        nc.vector.memset(h1b[:, :, :, 0:1], 0.0)
        nc.vector.memset(h1b[:, :, :, 33:35], 0.0)
        for c in range(4):
            pslice = slice(32 * c, 32 * c + 32)
            nc.vector.tensor_tensor(
                h1b[pslice, :, :, 1:33],
                h1pre[pslice, :, :, :, 0:64:2].rearrange(
                    "p a b h w -> p (a b) h w"
                ),
                h1pre[pslice, :, :, :, 1:64:2].rearrange(
                    "p a b h w -> p (a b) h w"
                ),
                ALU.max,
            )

        # ---- stage C: conv2 (4 row-tiles x 2 col-slots) -> q2 in {0,1} ----
        # q2 layout: partition half = sample-subgroup, f slot = 8j+2c+i for
        # sample n = 16j + 4t + c (t = 2m+i); conv3 reads L/H halves as two
        # concurrent row-tiles over the same f slots.
        q2 = big.tile([128, 64, 6, 35], F8, tag="q2")
        nc.vector.memset(q2[:, :, :, 0:1], 0.5)
        nc.vector.memset(q2[:, :, :, 33:35], 0.5)
        for j in range(8):
            pp2 = [
                psum.tile([128, 384], F32, tag="pp", name=f"pp2_{j}_{c}")
                for c in range(4)
            ]
            if DR_CONV2:
                # m=0 slot: DoubleRow tap pairs (t0,t1) + (t2,zero) at output
                # partition base 0 (the only base the DoubleRow ISA permits);
                # the rhs pair dim is a stride-1 overlapping w-window pair.
                k0 = 4 * j
                for pi, (w0, lo) in enumerate(((0, 0), (2, 128))):
                    for c in range(4):  # row-tiles, concurrent
                        for n in range(2):  # TENSOR3D: one matmul per sample
                            base = h1b[
                                32 * c : 32 * c + 32, k0 + n, :, w0 : w0 + 32
                            ]
                            rhs = bass.AP(
                                tensor=base.tensor,
                                offset=base.offset,
                                ap=[base.ap[0], [1, 2]] + list(base.ap[1:]),
                            )
                            nc.tensor.matmul(
                                pp2[c][0:64, 192 * n : 192 * n + 192],
                                lhsT=w2l[
                                    32 * c : 32 * c + 32, lo : lo + 128
                                ].rearrange("k (two m) -> k two m", two=2),
                                rhs=rhs,
                                start=(pi == 0),
                                stop=(pi == 1),
                                perf_mode=mybir.MatmulPerfMode.DoubleRow,
                                tile_position=(32 * c, 0),
                            )
                mslots = (1,)
            else:
                mslots = (0, 1)
            # remaining slots: regular 3-tap accumulation (DoubleRow cannot
            # write output partition base 64)
            for m in mslots:
                k1 = 4 * j + 2 * m
                for t in range(3):
                    for c in range(4):
                        nc.tensor.matmul(
                            pp2[c][64 * m : 64 * m + 64],
                            lhsT=w2l[
                                32 * c : 32 * c + 32, t * 64 : (t + 1) * 64
                            ],
                            rhs=h1b[
                                32 * c : 32 * c + 32, k1 : k1 + 2, :, t : t + 32
                            ],
                            start=(t == 0),
                            stop=(t == 2),
                            tile_position=(32 * c, 64 * m),
                        )
            for c in range(4):
                nc.vector.tensor_scalar(
                    q2[:, 8 * j + 2 * c : 8 * j + 2 * c + 2, :, 1:33],
                    pp2[c].rearrange("p (n h w) -> p n h w", n=2, h=6),
                    thr2t,
                    None,
                    ALU.is_ge,
                )

        # ---- stage D: conv3 (2 row-tiles) -> Sign -> pool -> h3b ----------
        h3pre = big.tile([128, 128, 6, 32], F8, tag="h3pre")
        for rnd in range(32):  # 4 samples per round via L/H row-tiles
            j, c = rnd // 4, rnd % 4
            s0 = 8 * j + 2 * c
            pp3 = [
                psum.tile([128, 384], F32, tag="pp", name=f"pp3_{rnd}_{g}")
                for g in range(2)
            ]
            if DR_CONV3:
                # the PE rhs pattern is TENSOR3D (3 free dims), so the pair
                # dim forces one matmul per sample (out at col offset 192*n)
                for pi, (w0, lo) in enumerate(((0, 0), (2, 256))):
                    for g in range(2):  # row-tile halves, concurrent
                        for n in range(2):
                            base = q2[
                                64 * g : 64 * g + 64, s0 + n, :, w0 : w0 + 32
                            ]
                            rhs = bass.AP(
                                tensor=base.tensor,
                                offset=base.offset,
                                ap=[base.ap[0], [1, 2]] + list(base.ap[1:]),
                            )
                            nc.tensor.matmul(
                                pp3[g][:, 192 * n : 192 * n + 192],
                                lhsT=w3l[
                                    64 * g : 64 * g + 64, lo : lo + 256
                                ].rearrange("k (two m) -> k two m", two=2),
                                rhs=rhs,
                                start=(pi == 0),
                                stop=(pi == 1),
                                perf_mode=mybir.MatmulPerfMode.DoubleRow,
                                tile_position=(64 * g, 0),
                            )
            else:
                for t in range(3):
                    for g in range(2):
                        nc.tensor.matmul(
                            pp3[g],
                            lhsT=w3l[
                                64 * g : 64 * g + 64,
                                t * 128 : (t + 1) * 128,
                            ],
                            rhs=q2[
                                64 * g : 64 * g + 64, s0 : s0 + 2, :, t : t + 32
                            ],
                            start=(t == 0),
                            stop=(t == 2),
                            tile_position=(64 * g, 0),
                        )
            for g in range(2):
                # samples {16j+c+8g, 16j+c+8g+4} -> strided n slice
                na = 16 * j + c + 8 * g
                nc.scalar.activation(
                    h3pre[:, na : na + 5 : 4],
                    pp3[g].rearrange("p (n h w) -> p n h w", n=2, h=6),
                    ACTF.Sign,
                    bias=thr3n,
                )
        h3b = big.tile([128, 128, 6, 16], F8, tag="h3b")
        for g in range(2):
            nc.vector.tensor_tensor(
                h3b[:, 64 * g : 64 * g + 64],
                h3pre[:, 64 * g : 64 * g + 64, :, 0:32:2],
                h3pre[:, 64 * g : 64 * g + 64, :, 1:32:2],
                ALU.max,
            )

        # ---- stage E: conv4 + BN4 + hardtanh -> h4 (bf16) -----------------
        h4 = big.tile([128, 128, 16], BF16, tag="h4")
        for rnd in range(4):
            pp4 = psum.tile([128, 512], F32, tag="pp")
            if DR_CONV4:
                for hp in range(3):  # DoubleRow h-tap pairs (2hp, 2hp+1)
                    base = h3b[:, 32 * rnd : 32 * rnd + 32, 2 * hp, :]
                    rhs = bass.AP(
                        tensor=base.tensor,
                        offset=base.offset,
                        ap=[base.ap[0], [16, 2]] + list(base.ap[1:]),
                    )
                    nc.tensor.matmul(
                        pp4,
                        lhsT=w4l[:, 256 * hp : 256 * hp + 256].rearrange(
                            "k (two m) -> k two m", two=2
                        ),
                        rhs=rhs,
                        start=(hp == 0),
                        stop=(hp == 2),
                        perf_mode=mybir.MatmulPerfMode.DoubleRow,
                    )
            else:
                for hh in range(6):
                    nc.tensor.matmul(
                        pp4,
                        lhsT=w4l[:, hh * 128 : (hh + 1) * 128],
                        rhs=h3b[:, 32 * rnd : 32 * rnd + 32, hh, :],
                        start=(hh == 0),
                        stop=(hh == 5),
                    )
            t4 = small.tile([128, 512], F32, tag="t4")
            nc.vector.tensor_scalar(t4, pp4, sc4t, bi4t, ALU.mult, ALU.add)
            nc.vector.tensor_scalar(
                h4[:, 32 * rnd : 32 * rnd + 32].rearrange("p n w -> p (n w)"),
                t4,
                1.0,
                -1.0,
                ALU.min,
                ALU.max,
            )

        # ---- stage F: FC (activation-stationary) + bias -------------------
        ppf = psum.tile([128, 16], F32, tag="pp")
        for w in range(16):
            nc.tensor.matmul(
                ppf[:, 0:10],
                lhsT=h4[:, :, w : w + 1],
                rhs=pk16[:, w * 10 : (w + 1) * 10],
                start=(w == 0),
                stop=(w == 15),
            )
        osb = small.tile([128, 10], BF16, tag="osb")
        nc.vector.tensor_tensor(osb, ppf[:, 0:10], bfct, ALU.add)
        nc.sync.dma_start(out=out[n0c : n0c + B], in_=osb)
    ctx.close()


# ---------------------------------------------------------------------------
# entry point: cached jitted shard_map executable
# ---------------------------------------------------------------------------
_RUNNER = None


class _Runner:
    def __init__(self):
        import jax
        from jax.sharding import Mesh, PartitionSpec
        from jax.experimental.shard_map import shard_map
        import concourse.bass2jax as b2j

        self.jax = jax
        nc = build_program()
        self.nc = nc
        b2j.install_neuronx_cc_hook()
        assert nc.dbg_addr is None and not getattr(nc, "dbg_callbacks", None)
        partition_name = (
            nc.partition_id_tensor.name if nc.partition_id_tensor else None
        )
        in_names, out_names, out_avals, self.out_shapes = [], [], [], []
        for alloc in nc.m.functions[0].allocations:
            if not isinstance(alloc, mybir.MemoryLocationSet):
                continue
            name = alloc.memorylocations[0].name
            if alloc.kind == "ExternalInput":
                if name != partition_name:
                    in_names.append(name)
            elif alloc.kind == "ExternalOutput":
                out_names.append(name)
                shape = tuple(alloc.tensor_shape)
                dtype = mybir.dt.np(alloc.dtype)
                out_avals.append(jax.core.ShapedArray(shape, dtype))
                self.out_shapes.append((shape, dtype))
        self.in_names = list(in_names)
        n_params = len(in_names)
        n_outs = len(out_avals)
        in_names_full = in_names + out_names
        if partition_name is not None:
            in_names_full.append(partition_name)

        def _body(*args):
            operands = list(args)
            if partition_name is not None:
                operands.append(b2j.partition_id_tensor())
            outs = b2j._bass_exec_p.bind(
                *operands,
                out_avals=tuple(out_avals),
                in_names=tuple(in_names_full),
                out_names=tuple(out_names),
                lowering_input_output_aliases=(),
                sim_require_finite=True,
                sim_require_nnan=True,
                nc=nc,
            )
            return tuple(outs)

        from jax.sharding import NamedSharding

        devices = jax.devices()[:N_CORES]
        assert len(devices) == N_CORES
        mesh = Mesh(np.asarray(devices), ("core",))
        self.spec = NamedSharding(mesh, PartitionSpec("core"))
        in_specs = (PartitionSpec("core"),) * (n_params + n_outs)
        out_specs = (PartitionSpec("core"),) * len(out_names)
        self.sharded = jax.jit(
            shard_map(
                _body,
                mesh=mesh,
                in_specs=in_specs,
                out_specs=out_specs,
                check_rep=False,
            ),
            keep_unused=True,
        )
        # The NEFF writes every element of every output, so the "output
        # seed" operands never need fresh zeros: keep one device-resident
        # copy and skip the per-call upload (no donation -> never mutated).
        self.d_zeros = [
            jax.device_put(np.zeros((N_CORES * s[0], *s[1:]), dt), self.spec)
            for s, dt in self.out_shapes
        ]
        self.param_key = None
        self.d_params = None


_PARAM_INPUTS = (
    "w1", "w2", "w3", "w4", "wfc", "bfc",
    "b1", "g1", "be1", "m1", "v1",
    "b2", "g2", "be2", "m2", "v2",
    "b3", "g3", "be3", "m3", "v3",
    "b4", "g4", "be4", "m4", "v4",
)


def _device_params(r, inputs):
    """Derived params live on device across calls (weights-stay-resident);
    recomputed + re-uploaded whenever any source tensor changes."""
    import zlib

    c, a = 0, 1
    for name in _PARAM_INPUTS:
        buf = np.ascontiguousarray(inputs[name]).data
        c = zlib.crc32(buf, c)
        a = zlib.adler32(buf, a)
    key = (c, a)
    if r.param_key != key:
        derived = host_prep(inputs)
        feed = {}
        for name, shape, _ in PARAM_SPECS:
            arr = derived[name]
            rep = np.broadcast_to(arr, (N_CORES, *arr.shape)).reshape(
                N_CORES * arr.shape[0], *arr.shape[1:]
            )
            feed[name] = r.jax.device_put(rep, r.spec)
        r.d_params = feed
        r.param_key = key
    return r.d_params


def kernel(**inputs):
    global _RUNNER
    if _RUNNER is None:
        _RUNNER = _Runner()
    r = _RUNNER

    feed = dict(_device_params(r, inputs))  # async uploads on cache miss
    feed["xq"] = host_pack_x(np.ascontiguousarray(inputs["x"], dtype=np.float32))
    args = [feed[name] for name in r.in_names] + r.d_zeros
    outs = r.sharded(*args)
    res = np.asarray(outs[0])
    return res.astype(np.float32)
